# revision 1
# baseline (speedup 1.0000x reference)
"""MHA kernel for Trainium2, 8 NeuronCores — fused-pipeline version.

Problem: B=4, T=2048, D=1024, H=16, HD=64 fp32 multi-head attention
  qkv = x @ w_qkv ; attention per head ; out = y @ w_o

Sharding: core c handles batch b = c//2 and head-group g = c%2 (8 of the 16
heads). Each core computes its 8 heads' attention output projected through
the matching w_o row-slice, producing a partial [T, D] output; the host sums
the two partials per batch (row-parallel output projection).

Single fused instruction stream, paced by the ACT engine's exp throughput
(the per-core floor: 8 heads x T^2 exps on 128 lanes @ 1.2 GHz ~ 274us).
All other engines hide inside it:
  - scores: two heads of a pair run CONCURRENTLY on the PE via row tiling
    (K=64 each, tile_position rows 0/64) -> pair costs ~N cycles, not 2N.
  - att@V: two heads run concurrently via col tiling (M=64 each, cols 0/64)
    into one [128, t] psum tile (rows 0-63 = even head, 64-127 = odd head),
    which is already y^T-oriented for the output projection.
  - softmax denominators: DVE accumulates exp tiles (f16 ping-pong), a
    1-column PE matmul (ones) does the partition reduction, DVE reciprocal,
    gpsimd partition_broadcast replicates 1/denom across partitions, one DVE
    multiply normalizes straight into yt (y^T, f16).
  - QKV projection / x-transposes / output projection are emitted as "fill"
    chains in the PE slack between score/att@V matmuls, through a 2-buffer
    [128,512] psum tag, so the PE never idles and stays at 2.4 GHz.
"""
import sys

if "/opt/trn_rl_repo" not in sys.path:
    sys.path.insert(0, "/opt/trn_rl_repo")

from collections import deque

import numpy as np

import concourse.bass as bass
import concourse.mybir as mybir
import concourse.tile as tile
from concourse import bacc
from concourse.bass_utils import run_bass_kernel_spmd
from concourse.masks import make_identity

T = 2048
D = 1024
NH = 8          # heads per core
HD = 64
KC = D // 128   # 8 contraction chunks
TT = T // 128   # 16 t/s tiles
NP = NH // 2    # 4 head pairs
F32 = mybir.dt.float32
F16 = mybir.dt.float16

_CACHE = {}


def build_nc():
    nc = bacc.Bacc(
        "TRN2",
        target_bir_lowering=False,
        debug=False,
        enable_asserts=False,
        num_devices=8,
    )
    x_d = nc.dram_tensor("x", [T, D], F16, kind="ExternalInput")
    wq_d = nc.dram_tensor("wq", [D, 512], F16, kind="ExternalInput")
    wk_d = nc.dram_tensor("wk", [D, 512], F16, kind="ExternalInput")
    wv_d = nc.dram_tensor("wv", [D, 512], F16, kind="ExternalInput")
    wo_d = nc.dram_tensor("wo", [512, D], F16, kind="ExternalInput")
    out_d = nc.dram_tensor("out", [T, D], F32, kind="ExternalOutput")

    x_ap = x_d.ap()
    wq_ap = wq_d.ap().rearrange("(kc p) j -> p kc j", p=128)   # [128, 8, 512]
    wk_ap = wk_d.ap().rearrange("(kc p) j -> p kc j", p=128)
    wv_ap = wv_d.ap().rearrange("(kc p) j -> p kc j", p=128)
    wo_ap = wo_d.ap().rearrange("(c p) n -> p c n", p=128)     # [128, 4, 1024]

    with tile.TileContext(nc) as tc:
        with (
            tc.sbuf_pool(name="sb", bufs=1) as sb,
            tc.psum_pool(name="ps", bufs=1) as ps,
        ):
            # ---- persistent sbuf ----
            xt = sb.tile([128, KC, T], F16)          # x^T  [d, t]
            qkt = sb.tile([128, 8, T], F16)          # jt 0-3 Q^T, 4-7 K^T
            v_sb = sb.tile([128, TT, 512], F16)      # V [s-part, s-chunk, j]
            yt = sb.tile([128, NP, T], F16)          # y^T [dy, pair, t]
            wqk_sb = sb.tile([128, KC, 1024], F16)   # cols 0-511 wq, 512+ wk
            wv_sb = sb.tile([128, KC, 512], F16)
            wo_sb = sb.tile([128, 4, D], F16)
            ones_v = sb.tile([128, 1], F16)
            nc.vector.memset(ones_v, 1.0)
            warm = sb.tile([1, 32], F16)
            nc.vector.memset(warm, 0.0)
            # warm up the ACT exp table before the stream needs it
            nc.scalar.activation(
                warm, warm, mybir.ActivationFunctionType.Exp, scale=0.125
            )

            nc.sync.dma_start(out=wqk_sb[:, :, 512:1024], in_=wk_ap)

            # ---------- chain emitters (each = one aux-psum chain) ----------
            def qk_chain(jt, tbc):
                """qkt[:, jt, tbc*512:(tbc+1)*512] = (w chunk)^T @ xt."""
                aux = ps.tile([128, 512], F32, tag="aux", bufs=2)
                for kc in range(KC):
                    nc.tensor.matmul(
                        aux,
                        wqk_sb[:, kc, jt * 128:(jt + 1) * 128],
                        xt[:, kc, tbc * 512:(tbc + 1) * 512],
                        start=(kc == 0),
                        stop=(kc == KC - 1),
                    )
                nc.vector.tensor_copy(
                    out=qkt[:, jt, tbc * 512:(tbc + 1) * 512], in_=aux
                )

            def v_chain(p, i):
                """v_sb[:, i, 128p:128p+128] = x-chunk @ wv cols."""
                aux = ps.tile([128, 512], F32, tag="aux", bufs=2)
                a = aux[:, 0:128]
                for kc in range(KC):
                    nc.tensor.matmul(
                        a,
                        xt[:, kc, i * 128:(i + 1) * 128],
                        wv_sb[:, kc, 128 * p:128 * p + 128],
                        start=(kc == 0),
                        stop=(kc == KC - 1),
                    )
                nc.vector.tensor_copy(
                    out=v_sb[:, i, 128 * p:128 * p + 128], in_=a
                )

            def o_chain(tt, u):
                """out[tt-block, u-half] = yt^T chunks @ wo."""
                aux = ps.tile([128, 512], F32, tag="aux", bufs=2)
                for c4 in range(4):
                    nc.tensor.matmul(
                        aux,
                        yt[:, c4, tt * 128:(tt + 1) * 128],
                        wo_sb[:, c4, u * 512:(u + 1) * 512],
                        start=(c4 == 0),
                        stop=(c4 == 3),
                    )
                o_sb = sb.tile([128, 512], F32, tag="osb", bufs=2)
                nc.vector.tensor_copy(out=o_sb, in_=aux)
                nc.sync.dma_start(
                    out=out_d.ap()[
                        tt * 128:(tt + 1) * 128, u * 512:(u + 1) * 512
                    ],
                    in_=o_sb,
                )

            # ---------- fill chains as generators (one MM per step) ----
            fills = deque()

            def g_qk(jt, tbc):
                aux = ps.tile([128, 512], F32, name="qkps",
                              tag="aux", bufs=2)
                for kc in range(KC):
                    nc.tensor.matmul(
                        aux,
                        wqk_sb[:, kc, jt * 128:(jt + 1) * 128],
                        xt[:, kc, tbc * 512:(tbc + 1) * 512],
                        start=(kc == 0),
                        stop=(kc == KC - 1),
                        skip_group_check=True,
                    )
                    yield 230
                nc.vector.tensor_copy(
                    out=qkt[:, jt, tbc * 512:(tbc + 1) * 512], in_=aux
                )

            def g_v(i):
                aux = ps.tile([128, 512], F32, name="vps",
                              tag="aux", bufs=2)
                for kc in range(KC):
                    nc.tensor.matmul(
                        aux,
                        xt[:, kc, i * 128:(i + 1) * 128],
                        wv_sb[:, kc, :],
                        start=(kc == 0),
                        stop=(kc == KC - 1),
                        skip_group_check=True,
                    )
                    yield 230
                nc.vector.tensor_copy(out=v_sb[:, i, :], in_=aux)

            def g_o(tt, u):
                aux = ps.tile([128, 512], F32, name="ops",
                              tag="aux", bufs=2)
                for c4 in range(4):
                    nc.tensor.matmul(
                        aux,
                        yt[:, c4, tt * 128:(tt + 1) * 128],
                        wo_sb[:, c4, u * 512:(u + 1) * 512],
                        start=(c4 == 0),
                        stop=(c4 == 3),
                        skip_group_check=True,
                    )
                    yield 230
                o_sb = sb.tile([128, 512], F32, tag="osb", bufs=2)
                nc.vector.tensor_copy(out=o_sb, in_=aux)
                nc.sync.dma_start(
                    out=out_d.ap()[
                        tt * 128:(tt + 1) * 128, u * 512:(u + 1) * 512
                    ],
                    in_=o_sb,
                )

            pending = {}

            def push_fill(key, gen):
                pending[key] = gen
                fills.append(key)

            def advance_fills(budget):
                while fills and budget > 0:
                    g = pending.get(fills[0])
                    if g is None:
                        fills.popleft()
                        continue
                    try:
                        budget -= next(g)
                    except StopIteration:
                        del pending[fills[0]]
                        fills.popleft()

            def need(key):
                g = pending.pop(key, None)
                if g is not None:
                    for _ in g:
                        pass

            def force_chain(g):
                for _ in g:
                    pass

            # ---------- attention stream state ----------
            sc_t = {
                "A": ps.tile([128, 1024], F32, name="sca", tag="sca", bufs=1),
                "B": ps.tile([128, 1024], F32, name="scb", tag="scb", bufs=1),
            }
            yu_t = [None]
            exp_t = {}
            acc_t = {}

            def emit_sc_pair(p, tb, iA, iB):
                for u in range(2):
                    for h, i in (("A", iA), ("B", iB)):
                        if i is None:
                            continue
                        pb = 0 if h == "A" else 64
                        nc.tensor.matmul(
                            sc_t[h][:, u * 512:(u + 1) * 512],
                            qkt[pb:pb + 64, 4 + p, i * 128:(i + 1) * 128],
                            qkt[pb:pb + 64, p,
                                tb * 1024 + u * 512:tb * 1024 + (u + 1) * 512],
                            start=True,
                            stop=True,
                        )

            def emit_exp(h, i):
                e = sb.tile([128, 1024], F16, tag="exp", bufs=8)
                nc.scalar.activation(
                    e, sc_t[h], mybir.ActivationFunctionType.Exp, scale=0.125
                )
                exp_t[(h, i)] = e

            def emit_acc(h, i):
                a = sb.tile([128, 1024], F16, tag="acc" + h, bufs=2)
                if i == 0:
                    nc.vector.tensor_copy(out=a, in_=exp_t[(h, i)])
                else:
                    with nc.allow_low_precision(reason="f16 exp-sum"):
                        nc.vector.tensor_add(
                            out=a, in0=acc_t[h], in1=exp_t[(h, i)]
                        )
                acc_t[h] = a

            def emit_yu_pair(p, iA, iB):
                for u in range(2):
                    for h, i in (("A", iA), ("B", iB)):
                        if i is None:
                            continue
                        pb = 0 if h == "A" else 64
                        nc.tensor.matmul(
                            yu_t[0][pb:pb + 64, u * 512:(u + 1) * 512],
                            v_sb[:, i, 128 * p + pb:128 * p + pb + 64],
                            exp_t[(h, i)][:, u * 512:(u + 1) * 512],
                            start=(i == 0),
                            stop=(i == TT - 1),
                            skip_group_check=True,
                        )
                for h, i in (("A", iA), ("B", iB)):
                    if i is not None:
                        exp_t.pop((h, i))

            norm_st = {}

            def emit_norm_half(h, p, tb, yu, acc):
                """denominator -> recip -> bcast -> normalize, one head."""
                hb = 0 if h == "A" else 1
                pb = 64 * hb
                if hb == 0:
                    norm_st["rec"] = sb.tile(
                        [1, 2048], F32, name="rec", tag="rec", bufs=1)
                    norm_st["bc"] = sb.tile(
                        [128, 2048], F32, name="recbc", tag="recbc", bufs=1)
                rec, bc = norm_st["rec"], norm_st["bc"]
                for u in range(2):
                    dn = ps.tile([128, 512], F32, name="dn",
                                 tag="aux", bufs=2)
                    nc.tensor.matmul(
                        dn[0:1, :],
                        ones_v,
                        acc[:, u * 512:(u + 1) * 512],
                        start=True,
                        stop=True,
                        tile_position=(0, 0),
                    )
                    nc.vector.reciprocal_approx_fast(
                        out=rec[0:1, 1024 * hb + 512 * u:
                                1024 * hb + 512 * (u + 1)],
                        in_=dn[0:1, :],
                    )
                nc.gpsimd.partition_broadcast(
                    bc[:, 1024 * hb:1024 * (hb + 1)],
                    rec[0:1, 1024 * hb:1024 * (hb + 1)],
                    channels=128,
                )
                with nc.allow_low_precision(reason="f16 y"):
                    for u in range(2):
                        nc.vector.tensor_mul(
                            out=yt[pb:pb + 64, p,
                                   tb * 1024 + u * 512:
                                   tb * 1024 + (u + 1) * 512],
                            in0=yu[pb:pb + 64, u * 512:(u + 1) * 512],
                            in1=bc[pb:pb + 64,
                                   1024 * hb + 512 * u:
                                   1024 * hb + 512 * (u + 1)],
                        )

            # ---------- startup: transpose-DMA x, minimal QK prefix ----
            for kc in range(KC):
                nc.sync.dma_start_transpose(
                    out=xt[:, kc, 0:1024],
                    in_=x_ap[0:1024, kc * 128:(kc + 1) * 128],
                )
            nc.sync.dma_start(out=wqk_sb[:, :, 0:512], in_=wq_ap)
            force_chain(g_qk(4, 0))   # K^T pair 0, s 0:512
            force_chain(g_qk(0, 0))   # Q^T pair 0, t 0:512
            force_chain(g_qk(0, 1))   # Q^T pair 0, t 512:1024
            nc.sync.dma_start(out=wv_sb, in_=wv_ap)
            v_gens = {i: g_v(i) for i in range(TT)}
            force_chain(v_gens.pop(0))
            force_chain(v_gens.pop(1))
            # remaining K^T chains stream into pair 0's window
            def xpose2(kc):
                nc.sync.dma_start_transpose(
                    out=xt[:, kc, 1024:2048],
                    in_=x_ap[1024:2048, kc * 128:(kc + 1) * 128],
                )

            startup_forced = {
                0: [lambda: [xpose2(kc) for kc in range(4)]],
                1: [lambda: [xpose2(kc) for kc in range(4, KC)],
                    lambda: force_chain(g_qk(4, 1))],
                2: [lambda: nc.sync.dma_start(out=wo_sb, in_=wo_ap)],
                4: [lambda: force_chain(g_qk(4, 2))],
                6: [lambda: force_chain(g_qk(4, 3))],
            }

            # ---------- fused attention stream (pair-outer) ----------
            deferred_norm = [None]

            for p in range(NP):
                for tb in range(2):
                    yu_t[0] = ps.tile(
                        [128, 1024], F32, name="yu", tag="yu", bufs=1
                    )
                    # force any producer chains this window consumes
                    if not (p == 0 and tb == 0):
                        for tbc in (2 * tb, 2 * tb + 1):
                            need(("qk", p, tbc))
                        for tbc in range(4):
                            need(("qk", 4 + p, tbc))
                    if p < 3:
                        jt = (p + 1) if tb == 0 else (4 + p + 1)
                        for tbc in range(4):
                            push_fill(("qk", jt, tbc), g_qk(jt, tbc))
                    if p == 0 and tb == 0:
                        push_fill(("qk", 0, 2), g_qk(0, 2))
                        push_fill(("qk", 0, 3), g_qk(0, 3))
                    if p == 3 and tb == 1:
                        for tt in range(8):
                            for u in range(2):
                                push_fill(("o", tt, u), g_o(tt, u))
                    first = (p == 0 and tb == 0)
                    for i in range(TT):
                        emit_sc_pair(p, tb, i, i - 1 if i >= 1 else None)
                        emit_exp("A", i)
                        if i == 0 and deferred_norm[0] is not None:
                            deferred_norm[0]()
                            deferred_norm[0] = None
                        if i >= 1:
                            emit_exp("B", i - 1)
                        emit_yu_pair(
                            p,
                            i - 1 if i >= 1 else None,
                            i - 2 if i >= 2 else None,
                        )
                        if first:
                            for fn in startup_forced.get(i, ()):
                                fn()
                            if i + 2 in v_gens:
                                force_chain(v_gens.pop(i + 2))
                        advance_fills(200 if first else 700)
                        emit_acc("A", i)
                        if i >= 1:
                            emit_acc("B", i - 1)
                    # tail slots
                    emit_sc_pair(p, tb, None, TT - 1)
                    emit_exp("B", TT - 1)
                    emit_yu_pair(p, TT - 1, TT - 2)
                    advance_fills(700)
                    emit_norm_half("A", p, tb, yu_t[0], acc_t["A"])
                    emit_acc("B", TT - 1)
                    emit_yu_pair(p, None, TT - 1)
                    deferred_norm[0] = (
                        lambda p=p, tb=tb, yu=yu_t[0], aB=acc_t["B"]:
                        emit_norm_half("B", p, tb, yu, aB)
                    )
            deferred_norm[0]()
            deferred_norm[0] = None

            # ---------- tail: output projection for tb=1 ----------
            while fills:
                advance_fills(10000)
            for tt in range(8, 16):
                for u in range(2):
                    force_chain(g_o(tt, u))

    nc.compile()
    return nc


def make_in_maps(x, w_qkv, w_o):
    in_maps = []
    for c in range(8):
        b, g = c // 2, c % 2
        in_maps.append({
            "x": np.ascontiguousarray(x[b], dtype=np.float16),
            "wq": np.ascontiguousarray(
                w_qkv[:, 512 * g:512 * (g + 1)], dtype=np.float16),
            "wk": np.ascontiguousarray(
                w_qkv[:, 1024 + 512 * g:1024 + 512 * (g + 1)],
                dtype=np.float16),
            "wv": np.ascontiguousarray(
                w_qkv[:, 2048 + 512 * g:2048 + 512 * (g + 1)],
                dtype=np.float16),
            "wo": np.ascontiguousarray(
                w_o[512 * g:512 * (g + 1), :], dtype=np.float16),
        })
    return in_maps


def kernel(x, w_qkv, w_o, _trace=False, _trace_kwargs=None):
    x = np.asarray(x)
    w_qkv = np.asarray(w_qkv)
    w_o = np.asarray(w_o)
    if "nc" not in _CACHE:
        _CACHE["nc"] = build_nc()
    nc = _CACHE["nc"]
    in_maps = make_in_maps(x, w_qkv, w_o)
    res = run_bass_kernel_spmd(
        nc, in_maps, core_ids=list(range(8)),
        trace=_trace, **(_trace_kwargs or {}),
    )
    out = np.empty((4, T, D), np.float32)
    for b in range(4):
        out[b] = res.results[2 * b]["out"] + res.results[2 * b + 1]["out"]
    if _trace:
        _CACHE["last_res"] = res
    return out

